# revision 1
# baseline (speedup 1.0000x reference)
"""GAT (3-layer, 4-head) graph-classification kernel for 8 Trainium2 NeuronCores.

Strategy (dst-sharded message passing):
  - Nodes are degree-sorted and dealt round-robin to 8 cores (graph/data parallel).
  - Per layer: each core computes h|al_src|al_dst for its node shard with one
    matmul (x_T @ [W | W@Asrc | W@Adst]), writes packed bf16 rows to a local
    HBM table shard, then an 8-core AllGather replicates the full node table.
  - Edges are sharded by destination. For tiles of 128 destination nodes,
    per-edge source rows are fetched with GPSIMD dma_gather (int16 indices,
    two gathers per group to cover >32767 row ids), attention softmax is done
    per destination on VectorE/ScalarE, messages are alpha-weighted in place
    and segment-summed along the free dimension.
  - Layer outputs are transposed back to feature-major (TensorE) to feed the
    next layer's matmul; after layer 3 a one-hot matmul pools node features
    into per-graph sums. Host sums the 8 per-core partial graph outputs.
"""

import sys

for _p in ("/opt/trn_rl_repo",):
    if _p not in sys.path:
        sys.path.insert(0, _p)

import numpy as np
import ml_dtypes

import concourse.bass as bass
import concourse.bacc as bacc
import concourse.mybir as mybir
import concourse.tile as tile
from concourse import library_config
from concourse.bass_utils import run_bass_kernel_spmd

FP = mybir.dt.float32
BF = mybir.dt.bfloat16
I16 = mybir.dt.int16
BFNP = ml_dtypes.bfloat16

# Problem constants (hardcoded per the harness contract).
N = 50000
E = 800000
IN = 128
H = 4
D = 64
HD = 256
G = 64
NEG = 0.2

NCORES = 8
TILES = 49                 # 128-node tiles per core
SHARD = TILES * 128        # 6272 rows per core (6250 real + 22 pad)
TOTROWS = NCORES * SHARD   # 50176
ROWW = 384                 # bf16 columns per table row (768 B): h[256] | al_src f32[4] | pad
SPLIT = 5 * TILES * 128    # 31360: region A = table rows of cores 0-4
CMAX = 40                  # max slot columns per tile-group
TMAX = 6                   # max 128-dst tiles per group
QB = 8                     # phase-A chunks per staging DMA
NEGINF = -1.0e30

_cache = {}


# ----------------------------------------------------------------------------
# Host-side preprocessing
# ----------------------------------------------------------------------------

def _preprocess(edge_index, batch):
    src = np.concatenate([edge_index[0], np.arange(N, dtype=np.int64)])
    dst = np.concatenate([edge_index[1], np.arange(N, dtype=np.int64)])
    deg = np.bincount(dst, minlength=N)

    # pass 1: deal nodes to cores by total-degree rank (load balance). A node's
    # table-row region is then fixed by its core (A = cores 0-4), which lets
    # pass 2 reorder freely within each core.
    order1 = np.argsort(-deg, kind="stable")
    core_of = np.empty(N, np.int64)
    core_of[order1] = np.arange(N) % NCORES
    srcA = core_of[src] < 5
    cntA = np.bincount(dst[srcA], minlength=N)
    degB = deg - cntA

    # pass 2: within each core sort by (degA, degB) desc so that each 128-node
    # tile is near-uniform in BOTH per-region degrees (minimal slot padding).
    core_nodes = np.full((NCORES, TILES * 128), -1, np.int64)
    node2row = np.full(N, -1, np.int64)
    for c in range(NCORES):
        nodes = order1[c::NCORES]
        nodes = nodes[np.lexsort((-degB[nodes], -cntA[nodes]))]
        core_nodes[c, : len(nodes)] = nodes  # index = t*128 + p (tile-major)
        t = np.arange(len(nodes)) // 128
        p = np.arange(len(nodes)) % 128
        node2row[nodes] = c * SHARD + p * TILES + t

    # CSR by destination with region-A (arow < SPLIT) edges first in each list
    arow_e = node2row[src]
    region = (arow_e >= SPLIT).astype(np.int64)
    eorder = np.lexsort((region, dst))
    arow_by = arow_e[eorder]
    dst_by = dst[eorder]
    starts = np.searchsorted(dst_by, np.arange(N))

    # per-tile slot widths, shared across cores for SPMD
    LA = np.zeros(TILES, np.int64)
    LB = np.zeros(TILES, np.int64)
    for t in range(TILES):
        nodes_t = core_nodes[:, t * 128 : (t + 1) * 128].reshape(-1)
        real = nodes_t >= 0
        if real.any():
            dA = cntA[nodes_t[real]]
            dT = deg[nodes_t[real]]
            LA[t] = dA.max()
            LB[t] = (dT - dA).max()

    groups = []
    t = 0
    while t < TILES:
        T = 1
        while T < TMAX and t + T < TILES:
            nLA = LA[t : t + T + 1].max()
            nLB = LB[t : t + T + 1].max()
            if (T + 1) * (nLA + nLB) <= CMAX:
                T += 1
            else:
                break
        groups.append((t, T, int(LA[t : t + T].max()), int(LB[t : t + T].max())))
        t += T

    tot_slots = sum(T * 128 * (gLA + gLB) for (_, T, gLA, gLB) in groups)
    n_edges = E + N

    # per-core packed idx / mask arrays
    XI = sum((T * gLA + T * gLB) * 8 for (_, T, gLA, gLB) in groups)
    XM = sum(T * (gLA + gLB) for (_, T, gLA, gLB) in groups)
    idx_all = np.zeros((NCORES, 128, XI), np.int16)
    mask_all = np.full((NCORES, 128, XM), NEGINF, np.float32)
    goffs = []  # (idx colA off, idx colB off, mask col off) per group

    arange128 = np.arange(128)
    for c in range(NCORES):
        io = 0
        mo = 0
        for gi, (t0, T, gLA, gLB) in enumerate(groups):
            if c == 0:
                goffs.append((io, io + T * gLA * 8, mo))
            CA, CB = T * gLA, T * gLB
            blkA = np.zeros((T * gLA, 128), np.int16)
            blkB = np.zeros((T * gLB, 128), np.int16)
            for ti in range(T):
                nodes_t = core_nodes[c, (t0 + ti) * 128 : (t0 + ti + 1) * 128]
                safe = np.maximum(nodes_t, 0)
                dA = np.where(nodes_t >= 0, cntA[safe], 0)
                dB = np.where(nodes_t >= 0, deg[safe] - cntA[safe], 0)
                st = starts[safe]
                if gLA:
                    ji = st[:, None] + np.arange(gLA)[None, :]
                    vals = arow_by[np.minimum(ji, n_edges - 1)]
                    valid = np.arange(gLA)[None, :] < dA[:, None]
                    vals = np.where(valid, vals, 0)
                    blkA[ti * gLA : (ti + 1) * gLA, :] = vals.T.astype(np.int16)
                    mask_all[c, :, mo + ti * gLA : mo + (ti + 1) * gLA] = np.where(
                        valid, 0.0, NEGINF
                    )
                if gLB:
                    ji = st[:, None] + dA[:, None] + np.arange(gLB)[None, :]
                    vals = arow_by[np.minimum(ji, n_edges - 1)] - SPLIT
                    valid = np.arange(gLB)[None, :] < dB[:, None]
                    vals = np.where(valid, vals, 0)
                    blkB[ti * gLB : (ti + 1) * gLB, :] = vals.T.astype(np.int16)
                    mask_all[
                        c, :, mo + CA + ti * gLB : mo + CA + (ti + 1) * gLB
                    ] = np.where(valid, 0.0, NEGINF)
            if gLA:
                w = blkA.reshape(-1).reshape(-1, 16).T  # [16, CA*8]
                idx_all[c, :, io : io + CA * 8] = np.tile(w, (8, 1))
                io += CA * 8
            if gLB:
                w = blkB.reshape(-1).reshape(-1, 16).T
                idx_all[c, :, io : io + CB * 8] = np.tile(w, (8, 1))
                io += CB * 8
            mo += CA + CB
        assert io == XI and mo == XM

    # pooling one-hot [p, t*G + g]
    onehot = np.zeros((NCORES, 128, TILES * G), np.float32)
    for c in range(NCORES):
        nodes = core_nodes[c]
        real = nodes >= 0
        tt = np.arange(TILES * 128) // 128
        pp = np.arange(TILES * 128) % 128
        gid = batch[np.maximum(nodes, 0)]
        onehot[c, pp[real], tt[real] * G + gid[real]] = 1.0

    return dict(
        core_nodes=core_nodes,
        groups=groups,
        goffs=goffs,
        idx_all=idx_all,
        mask_all=mask_all,
        onehot=onehot,
        XI=XI,
        XM=XM,
        tot_slots=tot_slots,
    )


def _build_wcat(W, a_src, a_dst):
    F = W.shape[0]
    Asrc = np.zeros((HD, H), np.float64)
    Adst = np.zeros((HD, H), np.float64)
    for h in range(H):
        Asrc[h * D : (h + 1) * D, h] = a_src[h]
        Adst[h * D : (h + 1) * D, h] = a_dst[h]
    Wc = np.zeros((F, 264), np.float64)
    Wc[:, 0:256] = W
    Wc[:, 256:260] = W @ Asrc
    Wc[:, 260:264] = W @ Adst
    return Wc.astype(BFNP)


# ----------------------------------------------------------------------------
# Bass program
# ----------------------------------------------------------------------------

def _build_program(meta, stage=3, repeat=1):
    groups = meta["groups"]
    goffs = meta["goffs"]
    XI, XM = meta["XI"], meta["XM"]
    CMAXG = max(T * (gLA + gLB) for (_, T, gLA, gLB) in groups)
    TMAXG = max(T for (_, T, _, _) in groups)

    nc = bacc.Bacc(
        "TRN2",
        target_bir_lowering=False,
        debug=False,
        enable_asserts=False,
        num_devices=NCORES,
    )

    d_x0T = nc.dram_tensor("x0T", [IN, SHARD], BF, kind="ExternalInput")
    d_wcat = [
        nc.dram_tensor(f"wcat{l}", [128 if l == 0 else 256, 264], BF, kind="ExternalInput")
        for l in range(3)
    ]
    d_bias = [
        nc.dram_tensor(f"bias{l}", [128, 256], FP, kind="ExternalInput") for l in range(3)
    ]
    d_ident = nc.dram_tensor("ident", [128, 128], BF, kind="ExternalInput")
    d_idx = nc.dram_tensor("idxall", [128, XI], I16, kind="ExternalInput")
    d_mask = nc.dram_tensor("maskall", [128, XM], FP, kind="ExternalInput")
    d_onehot = nc.dram_tensor("onehot", [128, TILES * G], FP, kind="ExternalInput")
    d_out = nc.dram_tensor("pooled", [G, HD], FP, kind="ExternalOutput")

    with tile.TileContext(nc) as tc:
        nc.gpsimd.load_library(library_config.mlp)
        with (
            tc.tile_pool(name="const", bufs=1) as cpool,
            tc.tile_pool(name="gath", bufs=3) as gpool,
            tc.tile_pool(name="att", bufs=2) as epool,
            tc.tile_pool(name="stage", bufs=2) as spool,
            tc.tile_pool(name="og", bufs=2) as ogpool,
            tc.tile_pool(name="psA", bufs=2, space="PSUM") as pspool,
            tc.tile_pool(name="psT", bufs=2, space="PSUM") as pstp,
            tc.tile_pool(name="psP", bufs=1, space="PSUM") as ppool,
            tc.tile_pool(name="dram", bufs=1, space="DRAM") as dpool,
        ):
            # resident tiles
            xT_a = cpool.tile([128, SHARD], BF, tag="xTa")
            xT_b = cpool.tile([128, SHARD], BF, tag="xTb")
            wcat_sb = []
            for l in range(3):
                ks = 1 if l == 0 else 2
                tiles_l = [
                    cpool.tile([128, 264], BF, name=f"wc{l}{k}", tag=f"wc{l}{k}")
                    for k in range(ks)
                ]
                wcat_sb.append(tiles_l)
            bias_sb = [cpool.tile([128, 256], FP, name=f"b{l}", tag=f"b{l}") for l in range(3)]
            ident = cpool.tile([128, 128], BF, tag="ident")
            idx_sb = cpool.tile([128, XI], I16, tag="idx")
            mask_sb = cpool.tile([128, XM], FP, tag="mask")
            onehot_sb = cpool.tile([128, TILES * G], FP, tag="oneh")
            aldst = cpool.tile([128, TILES * 4], FP, tag="aldst")

            tableshards = [
                dpool.tile(
                    [SHARD, ROWW], BF, name=f"tshard{lr}", tag=f"tshard{lr}"
                )
                for lr in range(3 * repeat)
            ]
            tablefulls = [
                dpool.tile(
                    [TOTROWS, ROWW],
                    BF,
                    name=f"tfull{lr}",
                    tag=f"tfull{lr}",
                    addr_space="Shared" if repeat == 1 else "Local",
                )
                for lr in range(3 * repeat)
            ]

            # constant loads
            nc.sync.dma_start(xT_a[:], d_x0T[:])
            for l in range(3):
                for k, wt in enumerate(wcat_sb[l]):
                    nc.sync.dma_start(wt[:], d_wcat[l][k * 128 : (k + 1) * 128, :])
                nc.sync.dma_start(bias_sb[l][:], d_bias[l][:])
            nc.sync.dma_start(ident[:], d_ident[:])
            nc.sync.dma_start(idx_sb[:], d_idx[:])
            nc.sync.dma_start(mask_sb[:], d_mask[:])
            nc.sync.dma_start(onehot_sb[:], d_onehot[:])

            nlayers = 2 if stage == 2 else (3 if stage == 3 else 1)
            reps = repeat
            for rep in range(reps):
              pool_ps = (
                ppool.tile([64, 256], FP, name="pool_ps", tag="poolps")
                if stage == 3
                else None
              )
              for l in range(nlayers):
                ks = 1 if l == 0 else 2
                tableshard = tableshards[rep * 3 + l]
                tablefull = tablefulls[rep * 3 + l]
                tsh3 = tableshard.rearrange("(p q) w -> p q w", q=TILES)
                tabA = tablefull[0:SPLIT, :]
                tabB = tablefull[SPLIT:TOTROWS, :]
                # ---------------- phase A: node transform + table shard ----
                for q0 in range(0, TILES, QB):
                    nq = min(QB, TILES - q0)
                    stg = spool.tile([128, QB * ROWW], BF, tag="stg")
                    stg3 = stg[:].rearrange("p (q w) -> p q w", w=ROWW)
                    stgf = stg[:].bitcast(FP).rearrange("p (q w) -> p q w", w=ROWW // 2)
                    nc.vector.memset(stg3[:, :, 264:384], 0)
                    for qi in range(nq):
                        q = q0 + qi
                        ps = pspool.tile([128, 264], FP, tag="psA")
                        nc.tensor.matmul(
                            ps[:],
                            xT_a[:, q * 128 : (q + 1) * 128],
                            wcat_sb[l][0][:],
                            start=True,
                            stop=(ks == 1),
                        )
                        if ks == 2:
                            nc.tensor.matmul(
                                ps[:],
                                xT_b[:, q * 128 : (q + 1) * 128],
                                wcat_sb[l][1][:],
                                start=False,
                                stop=True,
                            )
                        nc.scalar.copy(stg3[:, qi, 0:256], ps[:, 0:256])
                        nc.vector.tensor_copy(stgf[:, qi, 128:132], ps[:, 256:260])
                        nc.vector.tensor_copy(
                            aldst[:, q * 4 : (q + 1) * 4], ps[:, 260:264]
                        )
                    nc.sync.dma_start(
                        tsh3[:, q0 : q0 + nq, :], stg3[:, 0:nq, :]
                    )

                # ---------------- allgather the packed node table ----------
                nc.gpsimd.collective_compute(
                    "AllGather",
                    mybir.AluOpType.bypass,
                    replica_groups=[list(range(NCORES))],
                    ins=[tableshard.opt()],
                    outs=[tablefull.opt()],
                )

                # ---------------- edge phase -------------------------------
                if stage == 0:
                    continue
                estage = stage if stage >= 10 else 99
                for gi, (t0, T, gLA, gLB) in enumerate(groups):
                    ioA, ioB, mo = goffs[gi]
                    CA, CB = T * gLA, T * gLB
                    C = CA + CB

                    # probe stages: 20 = 512B rows, 21 = 256B rows (timing only)
                    if stage in (20, 21):
                        ew = 256 if stage == 20 else 128
                        hxp = gpool.tile([128, CMAXG * ew], BF, tag="hxp")
                        hxp3 = hxp[:].rearrange("p (c w) -> p c w", w=ew)

                        def probe_gather(col0, ncols, tab, io):
                            for k0 in range(0, ncols, 8):
                                kc = min(8, ncols - k0)
                                nc.gpsimd.dma_gather(
                                    hxp3[:, col0 + k0 : col0 + k0 + kc, :],
                                    tab[:, 0:ew],
                                    idx_sb[:, io + k0 * 8 : io + (k0 + kc) * 8],
                                    kc * 128,
                                    kc * 128,
                                    ew,
                                    elem_step=ROWW,
                                )

                        if gLA:
                            probe_gather(0, CA, tabA, ioA)
                        if gLB:
                            probe_gather(CA, CB, tabB, ioB)
                        e = epool.tile([128, CMAXG * 4], FP, tag="e")
                        nc.vector.tensor_copy(e[:, 0:64], hxp3[:, 0, 0:64])
                        continue

                    hx = gpool.tile([128, CMAXG * ROWW], BF, tag="hx")
                    hx3 = hx[:].rearrange("p (c w) -> p c w", w=ROWW)

                    # device limit: ≤1024 indices per dma_gather instruction
                    def chunked_gather(col0, ncols, tab, io):
                        for k0 in range(0, ncols, 8):
                            kc = min(8, ncols - k0)
                            nc.gpsimd.dma_gather(
                                hx3[:, col0 + k0 : col0 + k0 + kc, :],
                                tab,
                                idx_sb[:, io + k0 * 8 : io + (k0 + kc) * 8],
                                kc * 128,
                                kc * 128,
                                ROWW,
                            )

                    if gLA:
                        chunked_gather(0, CA, tabA, ioA)
                    if gLB:
                        chunked_gather(CA, CB, tabB, ioB)

                    hxf = hx[:].bitcast(FP).rearrange("p (c w) -> p c w", w=ROWW // 2)
                    # alS[p, c, h] at f32 columns 128..132 of each row
                    e = epool.tile([128, CMAXG * 4], FP, tag="e")
                    if estage == 10:
                        nc.vector.tensor_copy(e[:, 0:64], hx3[:, 0, 0:64])
                        continue
                    e3 = e[:].rearrange("p (c h) -> p c h", h=4)
                    ab = epool.tile([128, CMAXG * 4], BF, tag="ab")
                    ab3 = ab[:].rearrange("p (c h) -> p c h", h=4)

                    alD = aldst[:].rearrange("p (t h) -> p t h", h=4)[
                        :, t0 : t0 + T, :
                    ]

                    def reg_view(v3, off, L):
                        # [p, c, x] cols off..off+T*L -> [p, T, L, x]
                        return v3[:, off : off + T * L, :].rearrange(
                            "p (t j) h -> p t j h", j=L
                        )

                    regions = []
                    if gLA:
                        regions.append((0, gLA))
                    if gLB:
                        regions.append((CA, gLB))

                    # logits: e = al_src[src] + al_dst[dst]
                    for off, L in regions:
                        alS_r = hxf[:, off : off + T * L, 128:132].rearrange(
                            "p (t j) h -> p t j h", j=L
                        )
                        alD_b = alD.unsqueeze(2).broadcast_to((128, T, L, 4))
                        nc.vector.tensor_add(reg_view(e3, off, L), alS_r, alD_b)

                    eflat = e[:, : C * 4]
                    # leaky relu (composed: e = max(e,0) + NEG*min(e,0)), then pad mask
                    lr = epool.tile([128, CMAXG * 4], FP, name="lr", tag="lr")
                    lrf = lr[:, : C * 4]
                    nc.vector.tensor_scalar_min(lrf, eflat, 0.0)
                    nc.vector.tensor_scalar_max(eflat, eflat, 0.0)
                    nc.vector.scalar_tensor_tensor(
                        eflat,
                        lrf,
                        NEG,
                        eflat,
                        op0=mybir.AluOpType.mult,
                        op1=mybir.AluOpType.add,
                    )
                    mask_b = (
                        mask_sb[:, mo : mo + C].unsqueeze(2).broadcast_to((128, C, 4))
                    )
                    nc.vector.tensor_add(e3[:, 0:C, :], e3[:, 0:C, :], mask_b)

                    # segment max
                    mt = []
                    for off, L in regions:
                        m_r = epool.tile([128, TMAXG * 4], FP, name=f"m{off == 0}", tag=f"m{off == 0}")
                        in_r = (
                            e3[:, off : off + T * L, :]
                            .rearrange("p (t j) h -> p t h j", j=L)
                        )
                        nc.vector.reduce_max(
                            m_r[:, : T * 4], in_r, axis=mybir.AxisListType.X
                        )
                        mt.append(m_r)
                    if len(mt) == 2:
                        m = mt[0]
                        nc.vector.tensor_max(m[:, : T * 4], m[:, : T * 4], mt[1][:, : T * 4])
                    else:
                        m = mt[0]
                    m3 = m[:].rearrange("p (t h) -> p t h", h=4)[:, 0:T, :]

                    # ex = exp(e - m)
                    for off, L in regions:
                        m_b = m3.unsqueeze(2).broadcast_to((128, T, L, 4))
                        nc.vector.tensor_sub(
                            reg_view(e3, off, L), reg_view(e3, off, L), m_b
                        )
                    nc.scalar.activation(
                        eflat, eflat, mybir.ActivationFunctionType.Exp
                    )

                    # denom and reciprocal
                    dt_ = []
                    for off, L in regions:
                        d_r = epool.tile([128, TMAXG * 4], FP, name=f"d{off == 0}", tag=f"d{off == 0}")
                        in_r = (
                            e3[:, off : off + T * L, :]
                            .rearrange("p (t j) h -> p t h j", j=L)
                        )
                        nc.vector.reduce_sum(
                            d_r[:, : T * 4], in_r, axis=mybir.AxisListType.X
                        )
                        dt_.append(d_r)
                    den = dt_[0]
                    if len(dt_) == 2:
                        nc.vector.tensor_add(
                            den[:, : T * 4], den[:, : T * 4], dt_[1][:, : T * 4]
                        )
                    rec = epool.tile([128, TMAXG * 4], FP, tag="rec")
                    nc.vector.reciprocal(rec[:, : T * 4], den[:, : T * 4])
                    r3 = rec[:].rearrange("p (t h) -> p t h", h=4)[:, 0:T, :]

                    # alpha = ex / denom, cast to bf16
                    for off, L in regions:
                        r_b = r3.unsqueeze(2).broadcast_to((128, T, L, 4))
                        nc.vector.tensor_mul(
                            reg_view(e3, off, L), reg_view(e3, off, L), r_b
                        )
                    nc.vector.tensor_copy(ab[:, : C * 4], eflat)

                    if estage == 11:
                        continue
                    # messages: hx[:, :, 0:256] *= alpha (broadcast over 64)
                    h4 = hx3[:, 0:C, 0:256].rearrange("p c (h d) -> p c h d", d=D)
                    a4 = ab3[:, 0:C, :].unsqueeze(3).broadcast_to((128, C, 4, D))
                    nc.vector.tensor_mul(h4, h4, a4)

                    # segment sum -> [p, T, 256]
                    og = ogpool.tile([128, TMAXG * 256], FP, tag="ogA")
                    ogt = []
                    for off, L in regions:
                        o_r = (
                            og
                            if not ogt
                            else ogpool.tile(
                                [128, TMAXG * 256], FP, name="ogB", tag="ogB"
                            )
                        )
                        in_r = hx3[:, off : off + T * L, 0:256].rearrange(
                            "p (t j) f -> p t f j", j=L
                        )
                        nc.vector.reduce_sum(
                            o_r[:, : T * 256], in_r, axis=mybir.AxisListType.X
                        )
                        ogt.append(o_r)
                    if len(ogt) == 2:
                        nc.vector.tensor_add(
                            og[:, : T * 256], og[:, : T * 256], ogt[1][:, : T * 256]
                        )

                    # bias + relu
                    og3 = og[:].rearrange("p (t f) -> p t f", f=256)
                    bias_b = bias_sb[l][:].unsqueeze(1).broadcast_to((128, T, 256))
                    nc.vector.tensor_add(og3[:, 0:T, :], og3[:, 0:T, :], bias_b)
                    relu_f = ogpool.tile([128, TMAXG * 256], FP, tag="reluf")
                    nc.scalar.activation(
                        relu_f[:, : T * 256],
                        og[:, : T * 256],
                        mybir.ActivationFunctionType.Relu,
                    )

                    if estage == 12:
                        continue
                    if l < 2:
                        relu_b = ogpool.tile([128, TMAXG * 256], BF, tag="relub")
                        nc.vector.tensor_copy(
                            relu_b[:, : T * 256], relu_f[:, : T * 256]
                        )
                        rb3 = relu_b[:].rearrange("p (t f) -> p t f", f=256)
                        for ti in range(T):
                            for fb, xt in ((0, xT_a), (1, xT_b)):
                                pt = pstp.tile([128, 128], BF, tag="psT")
                                nc.tensor.transpose(
                                    pt[:],
                                    rb3[:, ti, fb * 128 : (fb + 1) * 128],
                                    ident[:],
                                )
                                nc.scalar.copy(
                                    xt[:, (t0 + ti) * 128 : (t0 + ti + 1) * 128],
                                    pt[:],
                                )
                    else:
                        rf3 = relu_f[:].rearrange("p (t f) -> p t f", f=256)
                        for ti in range(T):
                            q = t0 + ti
                            nc.tensor.matmul(
                                pool_ps[:],
                                onehot_sb[:, q * G : (q + 1) * G],
                                rf3[:, ti, :],
                                start=(q == 0),
                                stop=(q == TILES - 1),
                            )

            pout = cpool.tile([64, 256], FP, tag="pout")
            if stage == 3:
                nc.vector.tensor_copy(pout[:], pool_ps[:])
            else:
                nc.vector.memset(pout[:], 0.0)
                nc.vector.tensor_add(pout[:, 0:196], pout[:, 0:196], aldst[0:64, 0:196])
            nc.sync.dma_start(d_out[:], pout[:])

    nc.compile()
    return nc


# ----------------------------------------------------------------------------
# Entry point
# ----------------------------------------------------------------------------

def _prepare(inputs):
    key = (
        inputs["edge_index"].tobytes(),
        inputs["batch"].tobytes(),
    )
    kh = hash(key)
    if kh in _cache:
        return _cache[kh]
    edge_index = np.asarray(inputs["edge_index"], np.int64)
    batch = np.asarray(inputs["batch"], np.int64)
    meta = _preprocess(edge_index, batch)
    nc = _build_program(meta)
    _cache[kh] = (meta, nc)
    return meta, nc


def _make_inmaps(inputs, meta):
    x = np.asarray(inputs["x"], np.float32)
    batch = np.asarray(inputs["batch"], np.int64)
    core_nodes = meta["core_nodes"]

    wcats = []
    biases = []
    for l in range(3):
        Wl = np.asarray(inputs[f"W{l}"], np.float64)
        wcats.append(
            _build_wcat(
                Wl,
                np.asarray(inputs[f"a_src{l}"], np.float64),
                np.asarray(inputs[f"a_dst{l}"], np.float64),
            )
        )
        b = np.asarray(inputs[f"b{l}"], np.float32)
        biases.append(np.tile(b[None, :], (128, 1)).astype(np.float32))
    ident = np.eye(128, dtype=BFNP)

    in_maps = []
    for c in range(NCORES):
        nodes = core_nodes[c]
        safe = np.maximum(nodes, 0)
        x0 = x[safe]
        x0[nodes < 0] = 0.0
        # column q*128+p = node (tile q, partition p); core_nodes is tile-major
        x0T = np.ascontiguousarray(x0.T).astype(BFNP)
        in_maps.append(
            {
                "x0T": x0T,
                "wcat0": wcats[0],
                "wcat1": wcats[1],
                "wcat2": wcats[2],
                "bias0": biases[0],
                "bias1": biases[1],
                "bias2": biases[2],
                "ident": ident,
                "idxall": meta["idx_all"][c],
                "maskall": meta["mask_all"][c],
                "onehot": meta["onehot"][c],
            }
        )
    return in_maps


def _run(inputs, trace=False):
    meta, nc = _prepare(inputs)
    in_maps = _make_inmaps(inputs, meta)
    res = run_bass_kernel_spmd(
        nc, in_maps, core_ids=list(range(NCORES)), trace=trace
    )
    out = np.zeros((G, HD), np.float64)
    for c in range(NCORES):
        out += res.results[c]["pooled"].astype(np.float64)
    return out.astype(np.float32), res


def kernel(**inputs) -> np.ndarray:
    out, _ = _run(inputs, trace=False)
    return out


def kernel_traced(**inputs):
    out, res = _run(inputs, trace=True)
    return out, res



# revision 18
# speedup vs baseline: 1.8442x; 1.8442x over previous
"""GAT (3-layer, 4-head) graph-classification kernel for 8 Trainium2 NeuronCores.

Strategy (dst-sharded message passing, super-row gathers):
  - Nodes are degree-sorted and dealt round-robin to 8 cores (graph/data
    parallel); each core's nodes are laid out tile-major (49 tiles x 128).
  - Per layer: each core computes h|al_src|al_dst for its node shard with one
    matmul (x_T @ [W | W@Asrc | W@Adst]), writes packed bf16 640B rows
    (h[256] | al_src f32 | pad) to a local HBM table shard, then an 8-core
    AllGather replicates the full node table.
  - Edges are sharded by destination. Per-edge source rows are fetched with
    GPSIMD dma_gather at SUPER-row granularity: one 1280B descriptor covers a
    PAIR of adjacent table rows, halving descriptor count and keeping int16
    indices in range (25088 supers < 32767) with a single region. The
    wrong-half ("phantom") slot of each pair is killed via the attention mask
    (-inf logit => alpha=0). Gather descriptors round-robin 4 SWDGE queues.
  - Attention softmax per destination runs on VectorE/ScalarE over 2x virtual
    slots; messages are alpha-weighted in place and segment-summed along the
    free dimension.
  - Layer outputs are transposed back to feature-major (TensorE) to feed the
    next layer's matmul; after layer 3 a one-hot matmul pools node features
    into per-graph sums. Host sums the 8 per-core partial graph outputs.
"""

import sys

for _p in ("/opt/trn_rl_repo",):
    if _p not in sys.path:
        sys.path.insert(0, _p)

import numpy as np
import ml_dtypes

import concourse.bass as bass
import concourse.bacc as bacc
import concourse.mybir as mybir
import concourse.tile as tile
from concourse import library_config
from concourse.bass_utils import run_bass_kernel_spmd

FP = mybir.dt.float32
BF = mybir.dt.bfloat16
I16 = mybir.dt.int16
BFNP = ml_dtypes.bfloat16

# Problem constants (hardcoded per the harness contract).
N = 50000
E = 800000
IN = 128
H = 4
D = 64
HD = 256
G = 64
NEG = 0.2

NCORES = 8
TILES = 49                 # 128-node tiles per core
SHARD = TILES * 128        # 6272 rows per core (6250 real + 22 pad)
TOTROWS = NCORES * SHARD   # 50176
NSUP = TOTROWS // 2        # 25088 super-rows (fits int16)
ROWW = 320                 # bf16 columns per table row (640 B): h[256] | al_src f32[4] | pad
SUPW = 2 * ROWW            # 640 cols = 1280 B per gather descriptor
CMAX = 36                  # max super-slot columns per tile-group
TMAX = 6                   # max 128-dst tiles per group
QB = 8                     # phase-A chunks per staging DMA
NQUEUES = 4                # SWDGE queues for gather round-robin
NEGINF = -1.0e30

_cache = {}


# ----------------------------------------------------------------------------
# Host-side preprocessing
# ----------------------------------------------------------------------------

def _preprocess(edge_index, batch):
    src = np.concatenate([edge_index[0], np.arange(N, dtype=np.int64)])
    dst = np.concatenate([edge_index[1], np.arange(N, dtype=np.int64)])
    deg = np.bincount(dst, minlength=N)

    # deal nodes to cores by degree rank (load balance + uniform tile widths)
    order = np.argsort(-deg, kind="stable")
    core_nodes = np.full((NCORES, TILES * 128), -1, np.int64)
    node2row = np.full(N, -1, np.int64)
    for c in range(NCORES):
        nodes = order[c::NCORES]
        core_nodes[c, : len(nodes)] = nodes  # index = t*128 + p (tile-major)
        t = np.arange(len(nodes)) // 128
        p = np.arange(len(nodes)) % 128
        node2row[nodes] = c * SHARD + p * TILES + t

    # CSR by destination
    eorder = np.argsort(dst, kind="stable")
    row_by = node2row[src][eorder]
    dst_by = dst[eorder]
    starts = np.searchsorted(dst_by, np.arange(N))
    n_edges = E + N

    # per-tile slot widths, shared across cores for SPMD
    L = np.zeros(TILES, np.int64)
    for t in range(TILES):
        nodes_t = core_nodes[:, t * 128 : (t + 1) * 128].reshape(-1)
        real = nodes_t >= 0
        if real.any():
            L[t] = deg[nodes_t[real]].max()

    groups = []  # (t0, T, gL)
    t = 0
    while t < TILES:
        T = 1
        while (
            T < TMAX
            and t + T < TILES
            and (T + 1) * max(int(L[t : t + T].max()), int(L[t + T])) <= CMAX
        ):
            T += 1
        groups.append((t, T, int(L[t : t + T].max())))
        t += T

    tot_slots = sum(T * 128 * gL for (_, T, gL) in groups)

    # per-core packed idx / mask arrays
    XI = sum(T * gL * 8 for (_, T, gL) in groups)
    XM = sum(2 * T * gL for (_, T, gL) in groups)
    idx_all = np.zeros((NCORES, 128, XI), np.int16)
    mask_all = np.full((NCORES, 128, XM), NEGINF, np.float32)
    goffs = []  # (idx col off, mask col off) per group

    for c in range(NCORES):
        io = 0
        mo = 0
        for gi, (t0, T, gL) in enumerate(groups):
            if c == 0:
                goffs.append((io, mo))
            C = T * gL
            blk = np.zeros((C, 128), np.int16)
            for ti in range(T):
                nodes_t = core_nodes[c, (t0 + ti) * 128 : (t0 + ti + 1) * 128]
                safe = np.maximum(nodes_t, 0)
                dd = np.where(nodes_t >= 0, deg[safe], 0)
                st = starts[safe]
                ji = st[:, None] + np.arange(gL)[None, :]
                rows = row_by[np.minimum(ji, n_edges - 1)]
                valid = np.arange(gL)[None, :] < dd[:, None]
                rows = np.where(valid, rows, 0)
                sup = rows >> 1
                par = rows & 1
                blk[ti * gL : (ti + 1) * gL, :] = sup.T.astype(np.int16)
                # mask per virtual column: 2*(ti*gL + j) + half
                mslice = np.full((128, gL, 2), NEGINF, np.float32)
                okh = np.stack([(par == 0) & valid, (par == 1) & valid], axis=-1)
                mslice[okh] = 0.0
                mask_all[
                    c, :, mo + 2 * ti * gL : mo + 2 * (ti + 1) * gL
                ] = mslice.reshape(128, 2 * gL)
            w = blk.reshape(-1).reshape(-1, 16).T  # [16, C*8]
            idx_all[c, :, io : io + C * 8] = np.tile(w, (8, 1))
            io += C * 8
            mo += 2 * C
        assert io == XI and mo == XM

    # pooling one-hot [p, t*G + g]
    onehot = np.zeros((NCORES, 128, TILES * G), np.float32)
    for c in range(NCORES):
        nodes = core_nodes[c]
        real = nodes >= 0
        tt = np.arange(TILES * 128) // 128
        pp = np.arange(TILES * 128) % 128
        gid = batch[np.maximum(nodes, 0)]
        onehot[c, pp[real], tt[real] * G + gid[real]] = 1.0

    return dict(
        core_nodes=core_nodes,
        groups=groups,
        goffs=goffs,
        idx_all=idx_all,
        mask_all=mask_all,
        onehot=onehot,
        XI=XI,
        XM=XM,
        tot_slots=tot_slots,
    )


def _build_wcat(W, a_src, a_dst):
    F = W.shape[0]
    Asrc = np.zeros((HD, H), np.float64)
    Adst = np.zeros((HD, H), np.float64)
    for h in range(H):
        Asrc[h * D : (h + 1) * D, h] = a_src[h]
        Adst[h * D : (h + 1) * D, h] = a_dst[h]
    Wc = np.zeros((F, 264), np.float64)
    Wc[:, 0:256] = W
    Wc[:, 256:260] = W @ Asrc
    Wc[:, 260:264] = W @ Adst
    return Wc.astype(BFNP)


# ----------------------------------------------------------------------------
# Bass program
# ----------------------------------------------------------------------------

def _build_program(meta, stage=3, repeat=1, nqueues=NQUEUES):
    groups = meta["groups"]
    goffs = meta["goffs"]
    XI, XM = meta["XI"], meta["XM"]
    CSMAX = max(T * gL for (_, T, gL) in groups)       # super-slot columns
    CVMAX = 2 * CSMAX                                  # virtual slot columns
    TMAXG = max(T for (_, T, _) in groups)

    nc = bacc.Bacc(
        "TRN2",
        target_bir_lowering=False,
        debug=False,
        enable_asserts=False,
        num_devices=NCORES,
        num_swdge_queues=nqueues,
    )

    d_x0T = nc.dram_tensor("x0T", [IN, SHARD], BF, kind="ExternalInput")
    d_wcat = [
        nc.dram_tensor(f"wcat{l}", [128 if l == 0 else 256, 264], BF, kind="ExternalInput")
        for l in range(3)
    ]
    d_bias = [
        nc.dram_tensor(f"bias{l}", [128, 256], FP, kind="ExternalInput") for l in range(3)
    ]
    d_ident = nc.dram_tensor("ident", [128, 128], BF, kind="ExternalInput")
    d_idx = nc.dram_tensor("idxall", [128, XI], I16, kind="ExternalInput")
    d_mask = nc.dram_tensor("maskall", [128, XM], FP, kind="ExternalInput")
    d_onehot = nc.dram_tensor("onehot", [128, TILES * G], FP, kind="ExternalInput")
    d_out = nc.dram_tensor("pooled", [G, HD], FP, kind="ExternalOutput")

    with tile.TileContext(nc) as tc:
        nc.gpsimd.load_library(library_config.mlp)
        with (
            tc.tile_pool(name="const", bufs=1) as cpool,
            tc.tile_pool(name="gath", bufs=2) as gpool,
            tc.tile_pool(name="att", bufs=2) as epool,
            tc.tile_pool(name="stage", bufs=2) as spool,
            tc.tile_pool(name="og", bufs=2) as ogpool,
            tc.tile_pool(name="psA", bufs=2, space="PSUM") as pspool,
            tc.tile_pool(name="psT", bufs=2, space="PSUM") as pstp,
            tc.tile_pool(name="psP", bufs=1, space="PSUM") as ppool,
            tc.tile_pool(name="dram", bufs=1, space="DRAM") as dpool,
        ):
            # resident tiles
            xT_a = cpool.tile([128, SHARD], BF, tag="xTa")
            xT_b = cpool.tile([128, SHARD], BF, tag="xTb")
            wcat_sb = []
            for l in range(3):
                ks = 1 if l == 0 else 2
                tiles_l = [
                    cpool.tile([128, 264], BF, name=f"wc{l}{k}", tag=f"wc{l}{k}")
                    for k in range(ks)
                ]
                wcat_sb.append(tiles_l)
            bias_sb = [cpool.tile([128, 256], FP, name=f"b{l}", tag=f"b{l}") for l in range(3)]
            ident = cpool.tile([128, 128], BF, tag="ident")
            idx_sb = cpool.tile([128, XI], I16, tag="idx")
            mask_sb = cpool.tile([128, XM], FP, tag="mask")
            onehot_sb = cpool.tile([128, TILES * G], FP, tag="oneh")
            aldst = cpool.tile([128, TILES * 4], FP, tag="aldst")

            tableshards = [
                dpool.tile(
                    [SHARD, ROWW], BF, name=f"tshard{lr}", tag=f"tshard{lr}"
                )
                for lr in range(3 * repeat)
            ]
            tablefulls = [
                dpool.tile(
                    [TOTROWS, ROWW],
                    BF,
                    name=f"tfull{lr}",
                    tag=f"tfull{lr}",
                    addr_space="Shared",
                )
                for lr in range(3 * repeat)
            ]

            # constant loads
            nc.sync.dma_start(xT_a[:], d_x0T[:])
            for l in range(3):
                for k, wt in enumerate(wcat_sb[l]):
                    nc.sync.dma_start(wt[:], d_wcat[l][k * 128 : (k + 1) * 128, :])
                nc.sync.dma_start(bias_sb[l][:], d_bias[l][:])
            nc.sync.dma_start(ident[:], d_ident[:])
            nc.sync.dma_start(idx_sb[:], d_idx[:])
            nc.sync.dma_start(mask_sb[:], d_mask[:])
            nc.sync.dma_start(onehot_sb[:], d_onehot[:])

            nlayers = 2 if stage == 2 else (3 if stage == 3 else 1)
            for rep in range(repeat):
              pool_ps = (
                ppool.tile([64, 256], FP, name="pool_ps", tag="poolps")
                if stage == 3
                else None
              )
              for l in range(nlayers):
                ks = 1 if l == 0 else 2
                tableshard = tableshards[rep * 3 + l]
                tablefull = tablefulls[rep * 3 + l]
                tsh3 = tableshard.rearrange("(p q) w -> p q w", q=TILES)
                tabsup = tablefull.rearrange("(a b) w -> a (b w)", b=2)
                # ---------------- phase A: node transform + table shard ----
                for q0 in range(0, TILES, QB):
                    nq = min(QB, TILES - q0)
                    stg = spool.tile([128, QB * ROWW], BF, tag="stg")
                    stg3 = stg[:].rearrange("p (q w) -> p q w", w=ROWW)
                    stgf = stg[:].bitcast(FP).rearrange("p (q w) -> p q w", w=ROWW // 2)
                    nc.vector.memset(stg3[:, :, 264:ROWW], 0)
                    for qi in range(nq):
                        q = q0 + qi
                        ps = pspool.tile([128, 264], FP, tag="psA")
                        nc.tensor.matmul(
                            ps[:],
                            xT_a[:, q * 128 : (q + 1) * 128],
                            wcat_sb[l][0][:],
                            start=True,
                            stop=(ks == 1),
                        )
                        if ks == 2:
                            nc.tensor.matmul(
                                ps[:],
                                xT_b[:, q * 128 : (q + 1) * 128],
                                wcat_sb[l][1][:],
                                start=False,
                                stop=True,
                            )
                        nc.scalar.copy(stg3[:, qi, 0:256], ps[:, 0:256])
                        nc.vector.tensor_copy(stgf[:, qi, 128:132], ps[:, 256:260])
                        nc.vector.tensor_copy(
                            aldst[:, q * 4 : (q + 1) * 4], ps[:, 260:264]
                        )
                    nc.sync.dma_start(
                        tsh3[:, q0 : q0 + nq, :], stg3[:, 0:nq, :]
                    )

                # ---------------- allgather the packed node table ----------
                nc.gpsimd.collective_compute(
                    "AllGather",
                    mybir.AluOpType.bypass,
                    replica_groups=[list(range(NCORES))],
                    ins=[tableshard.opt()],
                    outs=[tablefull.opt()],
                )

                # ---------------- edge phase -------------------------------
                if stage == 0:
                    continue
                estage = stage if stage >= 10 else 99
                qctr = [0]
                for gi, (t0, T, gL) in enumerate(groups):
                    io, mo = goffs[gi]
                    C = T * gL       # super slots
                    CV = 2 * C       # virtual slots
                    jL = 2 * gL      # virtual slots per tile

                    hx = gpool.tile([128, CSMAX * SUPW], BF, tag="hx")
                    hxs = hx[:].rearrange("p (c w) -> p c w", w=SUPW)
                    hx3 = hx[:].rearrange("p (c w) -> p c w", w=ROWW)

                    # device limit: <=1024 indices per dma_gather instruction
                    for k0 in range(0, C, 8):
                        kc = min(8, C - k0)
                        nc.gpsimd.dma_gather(
                            hxs[:, k0 : k0 + kc, :],
                            tabsup[0:NSUP, 0:SUPW],
                            idx_sb[:, io + k0 * 8 : io + (k0 + kc) * 8],
                            kc * 128,
                            kc * 128,
                            SUPW,
                            queue_num=qctr[0] % nqueues,
                        )
                        qctr[0] += 1

                    e = epool.tile([128, CVMAX * 4], FP, tag="e")
                    if estage == 10:
                        nc.vector.reduce_max(
                            e[:, 0:1],
                            hx3[:, 0:CV, 0:1].rearrange("p c o -> p o c"),
                            axis=mybir.AxisListType.X,
                        )
                        continue

                    hxf = hx[:].bitcast(FP).rearrange("p (c w) -> p c w", w=ROWW // 2)
                    # alS[p, cv, h] at f32 columns 128..132 of each row
                    e3 = e[:].rearrange("p (c h) -> p c h", h=4)
                    ab = epool.tile([128, CVMAX * 4], BF, tag="ab")
                    ab3 = ab[:].rearrange("p (c h) -> p c h", h=4)

                    alD = aldst[:].rearrange("p (t h) -> p t h", h=4)[
                        :, t0 : t0 + T, :
                    ]

                    # logits: e = al_src[src] + al_dst[dst]
                    alS_r = hxf[:, 0:CV, 128:132].rearrange(
                        "p (t j) h -> p t j h", j=jL
                    )
                    alD_b = alD.unsqueeze(2).broadcast_to((128, T, jL, 4))
                    e4 = e3[:, 0:CV, :].rearrange("p (t j) h -> p t j h", j=jL)
                    nc.vector.tensor_add(e4, alS_r, alD_b)

                    eflat = e[:, : CV * 4]
                    # leaky relu (composed: e = max(e,0) + NEG*min(e,0)), then pad mask
                    lr = epool.tile([128, CVMAX * 4], FP, name="lr", tag="lr")
                    lrf = lr[:, : CV * 4]
                    nc.vector.tensor_scalar_min(lrf, eflat, 0.0)
                    nc.vector.tensor_scalar_max(eflat, eflat, 0.0)
                    nc.vector.scalar_tensor_tensor(
                        eflat,
                        lrf,
                        NEG,
                        eflat,
                        op0=mybir.AluOpType.mult,
                        op1=mybir.AluOpType.add,
                    )
                    mask_b = (
                        mask_sb[:, mo : mo + CV].unsqueeze(2).broadcast_to((128, CV, 4))
                    )
                    nc.vector.tensor_add(e3[:, 0:CV, :], e3[:, 0:CV, :], mask_b)

                    # segment max over virtual slots of each tile
                    m = epool.tile([128, TMAXG * 4], FP, name="m", tag="m")
                    in_m = e3[:, 0:CV, :].rearrange("p (t j) h -> p t h j", j=jL)
                    nc.vector.reduce_max(
                        m[:, : T * 4], in_m, axis=mybir.AxisListType.X
                    )
                    m3 = m[:].rearrange("p (t h) -> p t h", h=4)[:, 0:T, :]

                    # ex = exp(e - m)
                    m_b = m3.unsqueeze(2).broadcast_to((128, T, jL, 4))
                    nc.vector.tensor_sub(e4, e4, m_b)
                    nc.scalar.activation(
                        eflat, eflat, mybir.ActivationFunctionType.Exp
                    )

                    # denom and reciprocal
                    den = epool.tile([128, TMAXG * 4], FP, name="den", tag="den")
                    nc.vector.reduce_sum(
                        den[:, : T * 4],
                        e3[:, 0:CV, :].rearrange("p (t j) h -> p t h j", j=jL),
                        axis=mybir.AxisListType.X,
                    )
                    rec = epool.tile([128, TMAXG * 4], FP, tag="rec")
                    nc.vector.reciprocal(rec[:, : T * 4], den[:, : T * 4])
                    r3 = rec[:].rearrange("p (t h) -> p t h", h=4)[:, 0:T, :]

                    # alpha = ex / denom, cast to bf16
                    r_b = r3.unsqueeze(2).broadcast_to((128, T, jL, 4))
                    nc.vector.tensor_mul(e4, e4, r_b)
                    nc.vector.tensor_copy(ab[:, : CV * 4], eflat)

                    if estage == 11:
                        continue
                    # messages: hx[:, :, 0:256] *= alpha (broadcast over 64)
                    h4 = hx3[:, 0:CV, 0:256].rearrange("p c (h d) -> p c h d", d=D)
                    a4 = ab3[:, 0:CV, :].unsqueeze(3).broadcast_to((128, CV, 4, D))
                    nc.vector.tensor_mul(h4, h4, a4)

                    # segment sum -> [p, T, 256]
                    og = ogpool.tile([128, TMAXG * 256], FP, tag="og")
                    in_o = hx3[:, 0:CV, 0:256].rearrange(
                        "p (t j) f -> p t f j", j=jL
                    )
                    nc.vector.reduce_sum(
                        og[:, : T * 256], in_o, axis=mybir.AxisListType.X
                    )

                    # bias + relu
                    og3 = og[:].rearrange("p (t f) -> p t f", f=256)
                    bias_b = bias_sb[l][:].unsqueeze(1).broadcast_to((128, T, 256))
                    nc.vector.tensor_add(og3[:, 0:T, :], og3[:, 0:T, :], bias_b)
                    relu_f = ogpool.tile([128, TMAXG * 256], FP, tag="reluf")
                    nc.scalar.activation(
                        relu_f[:, : T * 256],
                        og[:, : T * 256],
                        mybir.ActivationFunctionType.Relu,
                    )

                    if estage == 12:
                        continue
                    if l < 2:
                        relu_b = ogpool.tile([128, TMAXG * 256], BF, tag="relub")
                        nc.vector.tensor_copy(
                            relu_b[:, : T * 256], relu_f[:, : T * 256]
                        )
                        rb3 = relu_b[:].rearrange("p (t f) -> p t f", f=256)
                        for ti in range(T):
                            for fb, xt in ((0, xT_a), (1, xT_b)):
                                pt = pstp.tile([128, 128], BF, tag="psT")
                                nc.tensor.transpose(
                                    pt[:],
                                    rb3[:, ti, fb * 128 : (fb + 1) * 128],
                                    ident[:],
                                )
                                nc.scalar.copy(
                                    xt[:, (t0 + ti) * 128 : (t0 + ti + 1) * 128],
                                    pt[:],
                                )
                    else:
                        rf3 = relu_f[:].rearrange("p (t f) -> p t f", f=256)
                        for ti in range(T):
                            q = t0 + ti
                            nc.tensor.matmul(
                                pool_ps[:],
                                onehot_sb[:, q * G : (q + 1) * G],
                                rf3[:, ti, :],
                                start=(q == 0),
                                stop=(q == TILES - 1),
                            )

            pout = cpool.tile([64, 256], FP, tag="pout")
            if stage == 3:
                nc.vector.tensor_copy(pout[:], pool_ps[:])
            else:
                nc.vector.memset(pout[:], 0.0)
                nc.vector.tensor_add(pout[:, 0:196], pout[:, 0:196], aldst[0:64, 0:196])
            nc.sync.dma_start(d_out[:], pout[:])

    nc.compile()
    return nc


# ----------------------------------------------------------------------------
# Entry point
# ----------------------------------------------------------------------------

def _prepare(inputs):
    key = (
        inputs["edge_index"].tobytes(),
        inputs["batch"].tobytes(),
    )
    kh = hash(key)
    if kh in _cache:
        return _cache[kh]
    edge_index = np.asarray(inputs["edge_index"], np.int64)
    batch = np.asarray(inputs["batch"], np.int64)
    meta = _preprocess(edge_index, batch)
    nc = _build_program(meta)
    _cache[kh] = (meta, nc)
    return meta, nc


def _make_inmaps(inputs, meta):
    x = np.asarray(inputs["x"], np.float32)
    batch = np.asarray(inputs["batch"], np.int64)
    core_nodes = meta["core_nodes"]

    wcats = []
    biases = []
    for l in range(3):
        Wl = np.asarray(inputs[f"W{l}"], np.float64)
        wcats.append(
            _build_wcat(
                Wl,
                np.asarray(inputs[f"a_src{l}"], np.float64),
                np.asarray(inputs[f"a_dst{l}"], np.float64),
            )
        )
        b = np.asarray(inputs[f"b{l}"], np.float32)
        biases.append(np.tile(b[None, :], (128, 1)).astype(np.float32))
    ident = np.eye(128, dtype=BFNP)

    in_maps = []
    for c in range(NCORES):
        nodes = core_nodes[c]
        safe = np.maximum(nodes, 0)
        x0 = x[safe]
        x0[nodes < 0] = 0.0
        # column q*128+p = node (tile q, partition p); core_nodes is tile-major
        x0T = np.ascontiguousarray(x0.T).astype(BFNP)
        in_maps.append(
            {
                "x0T": x0T,
                "wcat0": wcats[0],
                "wcat1": wcats[1],
                "wcat2": wcats[2],
                "bias0": biases[0],
                "bias1": biases[1],
                "bias2": biases[2],
                "ident": ident,
                "idxall": meta["idx_all"][c],
                "maskall": meta["mask_all"][c],
                "onehot": meta["onehot"][c],
            }
        )
    return in_maps


def _run(inputs, trace=False):
    meta, nc = _prepare(inputs)
    in_maps = _make_inmaps(inputs, meta)
    res = run_bass_kernel_spmd(
        nc, in_maps, core_ids=list(range(NCORES)), trace=trace
    )
    out = np.zeros((G, HD), np.float64)
    for c in range(NCORES):
        out += res.results[c]["pooled"].astype(np.float64)
    return out.astype(np.float32), res


def kernel(**inputs) -> np.ndarray:
    out, _ = _run(inputs, trace=False)
    return out


def kernel_traced(**inputs):
    out, res = _run(inputs, trace=True)
    return out, res
